# revision 52
# baseline (speedup 1.0000x reference)
"""Trainium2 Bass kernel for a 12-head attention module (B=4, S=1024, E=256, H=12,
per-head dim = E — the module quirk that makes per-head weight fusion possible).

Sharding: 8 cores = 4 batches x 2 head-groups (6 heads each).  Each core computes
its partial fc projection; the host sums the two partials per batch element.

Algebraic fusion (host precomputes, in float64):
  A_h^T = scale * Wk_h @ Wq_h^T   (E x E)  ->  q/k projections collapse:
      uT_h    = A_h @ x^T            [E, S]
      scoresT = uT ki-block contracted with xT   [s_k, s_q]
  C_h = Wv_h @ Wfc_h   (E x E)  ->  the fc layer disappears:
      w_h  = x @ C_h                 [S, E]
      out  = sum_h softmax(scores_h) @ w_h
  bv and bfc become an exact host-side constant row:  out += bv @ Wfc + bfc.

Precision split (validated against the 2e-2 rel-l2 gate by host fp8 emulation):
  * uT and scores matmuls: fp8-e4m3 DoubleRow (two 128-contractions per
    instruction at 0.5 cycles/row = 4x the fp32r rate).  wa is host-scaled
    x256 into e4m3 range; exp applies scale=1/256.  Path error alone ~6e-3.
  * w = x @ (64 C): three fp8 DoubleRow terms  x_hi*C_hi + (x/8)*(8 dC) +
    (8 dx)*(8 C)  — hi/lo residual split on BOTH operands keeps w at
    ~fp32-precision (adds ~1e-4) at 0.75x the fp32r matmul cost.
  * probs @ w (ctx) stays fp32r: fp8 on either side of that product alone
    measures ~2.6e-2 — over the gate.  Total measured error: 5.8e-3.

The additive causal mask folds into the scores PSUM as fp8 DoubleRow matmuls
(lhsT = [240*I | 0] pair, rhs = mask-block pair, adding 240*em = 256*mask to
the 256-scaled scores), so masked entries reach exp() at score-225 and
underflow to exactly 0.0 in fp32 — no elementwise mask work on any engine.

Softmax denominators: tiny free-2 fp32r matmuls (probs-block^T @ 64-column)
accumulate 64*D per head into a 1-bank psD tile; one DVE copy + reciprocal
puts 1/(64 D) in SBUF (walrus allows the fused DVE op only one PSUM tensor
operand, and its ISA has no divide).  Per-head normalize + head-sum is then
ONE fused DVE op per q-block:
  scalar_tensor_tensor(acc, ctx_psum, recip_sbuf, acc, mult, add)

PSUM (8 banks): scores/uT tiles 2x[128,1024] double-buffered (4), w psum
[128,2,256] (1), ctx [128,256] double-buffered (2), psD (1).  PSUM->SBUF
copies may only run on DVE/Act (walrus: Pool cannot touch PSUM), so they are
balanced across both and software-pipelined: ctx+normalize of head h-1 and
the w matmuls of head h fill the PE while Act exps head h's score tiles.

Cost-model result: 53456 ns/core (PE 42.5us busy, Act 40.8, DVE 40.2), vs
75411 ns for the all-fp32r baseline = 1.41x.
"""

import numpy as np

import concourse.mybir as mybir
import concourse.tile as tile
from concourse import bacc
from concourse.bass_utils import run_bass_kernel_spmd

# Problem constants
B, S, E, H = 4, 1024, 256, 12
P = 128
NCORES = 8
HPC = H // 2            # heads per core
EH = E * HPC            # 1536 = per-core head width
KS_E = E // P           # 2 contraction subtiles over E
ST = S // P             # 8 row-blocks of S
EW = E                  # w width (denominators live in psD, not a ones col)
TILE_W = 1024           # scores psum tile width (2 PSUM banks)
BANK_W = 512            # fp32 elements per PSUM bank

FP8 = mybir.dt.float8e4
F32R = mybir.dt.float32r
DR = mybir.MatmulPerfMode.DoubleRow

LAST_RESULTS = None     # BassKernelResults of the most recent run (for harness)


def _mask_structure(attention_mask):
    """Classify 128x128 blocks of maskT -> (structure, unique fp8 mask blocks).

    Returns struct = (spans, tiles, covers, tot, nuniq) where
      spans[ki]  = (qa, qb) tight non-skip q-extent (block aligned) or None
      tiles      = per scores-psum-tile: (used_cols,
                     [(c0, c1, ki, q0), ...] bank-aligned matmul segments,
                     [(cm, uid), ...] mask-matmul blocks at tile col cm)
      covers[m]  = tuple of ki whose span fully covers q-block m
      tot        = packed probs width
    and ublocks = [nuniq, P, P] fp8-encodable f32 array of 4*maskT blocks.
    """
    m = np.asarray(attention_mask, dtype=np.float64).reshape(S, S)   # [q, k]
    mT = m.T                                                         # [k, q]

    uniq: dict[bytes, int] = {}
    blocks = {}
    for ki in range(ST):
        for qj in range(ST):
            blk = mT[ki * P:(ki + 1) * P, qj * P:(qj + 1) * P]
            if (blk <= -1000.0).all():
                blocks[(ki, qj)] = "skip"
            elif (blk == 0.0).all():
                blocks[(ki, qj)] = "one"
            else:
                # mask-matmul adds 240*em to the 256-scaled scores psum, so
                # em = mask*256/240 makes the exp see s+mask; fully-masked
                # entries clip to -240 -> scores-225 -> exp underflows to 0.0
                enc = np.clip(blk * (256.0 / 240.0), -240.0, 240.0).astype(np.float32)
                blocks[(ki, qj)] = uniq.setdefault(enc.tobytes(), len(uniq))

    allmask = np.full((P, P), -240.0, np.float32)
    spans = []
    for ki in range(ST):
        non = [qj for qj in range(ST) if blocks[(ki, qj)] != "skip"]
        spans.append(None if not non else (non[0] * P, (non[-1] + 1) * P))

    # packed probs layout: concatenated spans
    probs_off, tot = [], 0
    for ki in range(ST):
        probs_off.append(tot)
        if spans[ki] is not None:
            tot += spans[ki][1] - spans[ki][0]

    # mask-matmul blocks: every non-"one" block inside a span (interior skips
    # get the all--448 block so exp underflows to 0)
    mask_mm = {}    # packed block col -> uid
    for ki in range(ST):
        if spans[ki] is None:
            continue
        qa, qb = spans[ki]
        for qj in range(qa // P, qb // P):
            bl = blocks[(ki, qj)]
            if bl == "one":
                continue
            if bl == "skip":
                bl = uniq.setdefault(allmask.tobytes(), len(uniq))
            mask_mm[probs_off[ki] + qj * P - qa] = bl

    # scores psum tiles: split packed cols at TILE_W, segments at BANK_W
    ntiles = (tot + TILE_W - 1) // TILE_W
    tiles = []
    for t in range(ntiles):
        t0, t1 = t * TILE_W, min((t + 1) * TILE_W, tot)
        segs, masks = [], []
        for ki in range(ST):
            if spans[ki] is None:
                continue
            qa, qb = spans[ki]
            s0, s1 = probs_off[ki], probs_off[ki] + (qb - qa)
            a, b = max(s0, t0), min(s1, t1)
            pos = a
            while pos < b:
                nxt = min(b, (pos // BANK_W + 1) * BANK_W)
                segs.append((pos - t0, nxt - t0, ki, qa + pos - s0))
                pos = nxt
        for cm, uid in mask_mm.items():
            if t0 <= cm < t1:
                masks.append((cm - t0, uid))
        tiles.append((t1 - t0, tuple(segs), tuple(masks)))

    covers = []
    for m_ in range(ST):
        ks = tuple(ki for ki in range(ST)
                   if spans[ki] is not None
                   and spans[ki][0] <= m_ * P and spans[ki][1] >= (m_ + 1) * P)
        assert ks, (
            "attention row-block with no unmasked keys is not supported "
            "(reference softmax of an all-masked row is uniform)")
        covers.append(ks)

    nuniq = max(len(uniq), 1)
    ublocks = np.zeros((nuniq, P, P), np.float32)
    for key, uid in uniq.items():
        ublocks[uid] = np.frombuffer(key, np.float32).reshape(P, P)

    struct = (tuple(spans), tuple(tiles), tuple(covers),
              tuple(probs_off), tot, nuniq)
    return struct, ublocks


def _build(struct):
    spans, tiles, covers, probs_off, tot, nuniq = struct
    f32 = mybir.dt.float32
    Exp = mybir.ActivationFunctionType.Exp
    Alu = mybir.AluOpType

    nc = bacc.Bacc("TRN2")
    xT8_d = nc.dram_tensor("xT8", (E, S), FP8, kind="ExternalInput")
    xhs_d = nc.dram_tensor("xhs8", (E, S), FP8, kind="ExternalInput")
    xls_d = nc.dram_tensor("xls8", (E, S), FP8, kind="ExternalInput")
    wa8_d = nc.dram_tensor("wa8", (E, EH), FP8, kind="ExternalInput")
    chi_d = nc.dram_tensor("chi8", (E, EH), FP8, kind="ExternalInput")
    chis_d = nc.dram_tensor("chis8", (E, EH), FP8, kind="ExternalInput")
    clos_d = nc.dram_tensor("clos8", (E, EH), FP8, kind="ExternalInput")
    em_d = nc.dram_tensor("emask", (nuniq + 3, P, P), FP8, kind="ExternalInput")
    dcol_d = nc.dram_tensor("dcol", (P, 2), F32R, kind="ExternalInput")
    y_d = nc.dram_tensor("y", (S, E), f32, kind="ExternalOutput")

    ntiles = len(tiles)

    with tile.TileContext(nc) as tc, \
            tc.tile_pool(name="singles", bufs=1) as singles, \
            tc.tile_pool(name="heads", bufs=2) as heads, \
            tc.tile_pool(name="psA", bufs=2, space="PSUM") as psA, \
            tc.tile_pool(name="psW", bufs=1, space="PSUM") as psW, \
            tc.tile_pool(name="psC", bufs=2, space="PSUM") as psC, \
            tc.tile_pool(name="psD", bufs=1, space="PSUM") as psD:

        # ---- resident tensors, DMA'd in first-use order on 3 queues.
        # front-split so the first uT matmul's 128-col lhsT / 512-col rhs
        # unblock on tiny transfers instead of the full tensors ----
        h0 = slice(0, E)
        wa8_r = wa8_d[:, :].rearrange("(ko p) n -> p ko n", p=P)
        wa8_sb = singles.tile([P, KS_E, EH], FP8)
        nc.sync.dma_start(wa8_sb[:, :, 0:P], wa8_r[:, :, 0:P])
        xT8_sb = singles.tile([P, KS_E, S], FP8)
        xT8_r = xT8_d[:, :].rearrange("(ko p) n -> p ko n", p=P)
        nc.scalar.dma_start(xT8_sb[:, :, 0:BANK_W], xT8_r[:, :, 0:BANK_W])
        nc.sync.dma_start(wa8_sb[:, :, P:E], wa8_r[:, :, P:E])
        em_sb = singles.tile([P, nuniq + 3, P], FP8)
        nc.sync.dma_start(em_sb, em_d[:, :, :].rearrange("u p q -> p u q"))
        idz_sb = em_sb[:, 0:2, :]
        dcol_sb = singles.tile([P, 2], F32R)
        nc.sync.dma_start(dcol_sb, dcol_d[:, :])
        nc.scalar.dma_start(xT8_sb[:, :, BANK_W:S], xT8_r[:, :, BANK_W:S])  # Act idle at start
        chi_sb = singles.tile([P, KS_E, EH], FP8)
        chis_sb = singles.tile([P, KS_E, EH], FP8)
        clos_sb = singles.tile([P, KS_E, EH], FP8)
        for sb, d in ((chi_sb, chi_d), (chis_sb, chis_d), (clos_sb, clos_d)):
            nc.sync.dma_start(sb[:, :, h0],
                              d[:, h0].rearrange("(ko p) n -> p ko n", p=P))
        # x hi/lo shifts, front 256 cols first (w st0/st1 of head 0)
        xhs_sb = singles.tile([P, KS_E, S], FP8)
        xls_sb = singles.tile([P, KS_E, S], FP8)
        xhs_r = xhs_d[:, :].rearrange("(ko p) n -> p ko n", p=P)
        xls_r = xls_d[:, :].rearrange("(ko p) n -> p ko n", p=P)
        nc.gpsimd.dma_start(xhs_sb[:, :, 0:2 * P], xhs_r[:, :, 0:2 * P])
        nc.gpsimd.dma_start(xls_sb[:, :, 0:2 * P], xls_r[:, :, 0:2 * P])
        nc.gpsimd.dma_start(xhs_sb[:, :, 2 * P:S], xhs_r[:, :, 2 * P:S])
        nc.gpsimd.dma_start(xls_sb[:, :, 2 * P:S], xls_r[:, :, 2 * P:S])
        # heads 1..5 weights coalesced: one DMA per tensor, not per (h, tensor)
        hr = slice(E, EH)
        for sb, d in ((wa8_sb, wa8_d), (chi_sb, chi_d),
                      (chis_sb, chis_d), (clos_sb, clos_d)):
            nc.sync.dma_start(
                sb[:, :, hr], d[:, hr].rearrange("(ko p) n -> p ko n", p=P))
        acc_sb = singles.tile([P, ST, E], f32)
        nc.gpsimd.memset(acc_sb, 0.0)   # h==0 stt adds into zeroed acc

        state = {}   # per-head live tiles

        def emit_uT(h, jns=None):
            if jns is None:
                jns = range(S // BANK_W)
            if h in state and "uT" in state[h]:
                uT8 = state[h]["uT"]
            else:
                uT8 = heads.tile([P, KS_E, S], FP8, tag="uT", name="uT8")
                state[h] = {"uT": uT8}
            # psum tile jn holds [t0-jn | t1-jn] side by side, so each copy
            # moves one jn-half of BOTH t rows in a single strided instr; the
            # jn=0 copy lands first (early score tiles read low uT cols, and
            # at startup xT8's high half is still in flight)
            for jn in jns:
                js = slice(jn * BANK_W, (jn + 1) * BANK_W)
                ps = psA.tile([P, TILE_W], f32, tag="mm1024", name="ps_u")
                for t in range(KS_E):
                    nc.tensor.matmul(
                        ps[:, t * BANK_W:(t + 1) * BANK_W],
                        wa8_sb[:, :, h * E + t * P: h * E + (t + 1) * P],
                        xT8_sb[:, :, js],
                        start=True, stop=True, perf_mode=DR,
                    )
                pr = ps[:, :].rearrange("p (t n) -> p t n", t=KS_E)
                if jn == 0:
                    # tile0 only needs uT cols 0:128 (ki=0) — land them first;
                    # jn1 goes to Act in parallel (it idles at head boundary)
                    nc.vector.tensor_copy(uT8[:, :, 0:P], pr[:, :, 0:P])
                    nc.vector.tensor_copy(
                        uT8[:, :, P:BANK_W], pr[:, :, P:BANK_W])
                else:
                    nc.scalar.copy(uT8[:, :, js], pr)

        def emit_w(h, g2):
            # g2 in 0..3: two st-blocks per psW tile (single-bank, bufs=1)
            if g2 == 0:
                ww = heads.tile([P, ST, EW], F32R, tag="w", name="ww")
                state[h]["w"] = ww
            ww = state[h]["w"]
            he = slice(h * E, (h + 1) * E)
            psw = psW.tile([P, 2, E], f32, tag="psw", name="ps_w")
            for j in range(2):
                st = 2 * g2 + j
                ss = slice(st * P, (st + 1) * P)
                # w = x@(64 C) as 3 fp8 DoubleRow terms:
                #   x_hi*C_hi + (x/8)*(8 dC) + (8 dx)*(8C)
                for i, (xs, cs) in enumerate(
                        ((xT8_sb, chi_sb), (xhs_sb, clos_sb), (xls_sb, chis_sb))):
                    nc.tensor.matmul(
                        psw[:, j, :], xs[:, :, ss], cs[:, :, he],
                        start=(i == 0), stop=(i == 2), perf_mode=DR,
                    )
            # psum->sbuf copies may only run on DVE or Act (walrus forbids
            # Pool reading PSUM); mostly Act, some DVE, to balance engines
            if g2 >= 1:
                nc.vector.tensor_copy(ww[:, 2 * g2:2 * g2 + 2, :E], psw)
            else:
                nc.scalar.copy(ww[:, 2 * g2:2 * g2 + 2, :E], psw)

        def emit_probs(h):
            state[h]["probs"] = heads.tile([P, tot], F32R, tag="probs", name="probs")

        def emit_score_tile(h, t):
            used, segs, masks = tiles[t]
            uT8 = state[h]["uT"]
            probs = state[h]["probs"]
            ps = psA.tile([P, TILE_W], f32, tag="mm1024", name="ps_s")
            for (c0, c1, ki, q0) in segs:
                seg_masks = [(cm, uid) for cm, uid in masks if c0 <= cm < c1]
                nc.tensor.matmul(
                    ps[:, c0:c1],
                    uT8[:, :, ki * P:(ki + 1) * P],
                    xT8_sb[:, :, q0:q0 + (c1 - c0)],
                    start=True, stop=(not seg_masks), perf_mode=DR,
                )
                for i, (cm, uid) in enumerate(seg_masks):
                    nc.tensor.matmul(
                        ps[:, cm:cm + P], idz_sb, em_sb[:, 2 + uid:4 + uid, :],
                        start=False, stop=(i == len(seg_masks) - 1),
                        perf_mode=DR,
                    )
            nc.scalar.activation(probs[:, t * TILE_W: t * TILE_W + used],
                                 ps[:, :used], Exp, scale=1.0 / 256)

        def emit_D(h, ms):
            # softmax denominators D(q)*64 for q-blocks ms into this head's
            # psD tile: free-2 fp32r matmuls against the 64-valued column pair
            # (walrus allows stt only one PSUM tensor input, so D must reach
            # SBUF separately from the ctx psum)
            if "psD" not in state[h]:
                state[h]["psD"] = psD.tile([P, 2 * ST], f32, tag="psd",
                                           name="ps_d")
                state[h]["dsb"] = heads.tile([P, 2 * ST], f32, tag="dsb",
                                             name="dsb")
                state[h]["rsb"] = heads.tile([P, 2 * ST], f32, tag="rsb",
                                             name="rsb")
            psd = state[h]["psD"]
            probs = state[h]["probs"]
            for m_ in ms:
                ks_list = covers[m_]
                last = len(ks_list) - 1
                for idx, ki in enumerate(ks_list):
                    qa = spans[ki][0]
                    off = probs_off[ki]
                    nc.tensor.matmul(
                        psd[:, 2 * m_:2 * m_ + 2],
                        probs[:, off + m_ * P - qa: off + (m_ + 1) * P - qa],
                        dcol_sb,
                        start=(idx == 0), stop=(idx == last),
                    )

        def emit_Dcopy(h, lo, hi):
            # psD -> sbuf, then reciprocal (DVE stt has no divide in ISA)
            nc.vector.tensor_copy(state[h]["dsb"][:, 2 * lo:2 * hi],
                                  state[h]["psD"][:, 2 * lo:2 * hi])
            nc.vector.reciprocal(state[h]["rsb"][:, 2 * lo:2 * hi],
                                 state[h]["dsb"][:, 2 * lo:2 * hi])

        def emit_ctx(h, m_):
            probs = state[h]["probs"]
            ww = state[h]["w"]
            ks_list = covers[m_]
            ps = psC.tile([P, EW], f32, tag="ctx", name="ps_c")
            last = len(ks_list) - 1
            for idx, ki in enumerate(ks_list):
                qa = spans[ki][0]
                off = probs_off[ki]
                nc.tensor.matmul(
                    ps,
                    probs[:, off + m_ * P - qa: off + (m_ + 1) * P - qa],
                    ww[:, ki, :],
                    start=(idx == 0), stop=(idx == last),
                )
            # fused normalize + head accumulate: acc = ctx / D (+ acc).
            # h==0 uses op1=bypass; in1 must still be readable, so point it at
            # the psum (acc is uninitialized until h==0 writes it).
            rsb = state[h]["rsb"]
            nc.vector.scalar_tensor_tensor(
                acc_sb[:, m_, :], ps, rsb[:, 2 * m_:2 * m_ + 1],
                acc_sb[:, m_, :], Alu.mult, Alu.add)
            if h == HPC - 1:
                nc.sync.dma_start(y_d[m_ * P:(m_ + 1) * P, :], acc_sb[:, m_, :])

        # ---- software-pipelined schedule: ctx of head h-1 fills the gaps
        # while the Act engine exps head h's scores ----
        for h in range(HPC):
            emit_uT(h, jns=[0] if h == 0 else None)
            emit_probs(h)
            if h == 0:
                # startup: tiles 0-2 need only uT's jn0 half, so tile0 runs
                # before xT8's high half has even arrived; w groups fill gaps
                for t in range(ntiles):
                    emit_score_tile(0, t)
                    if t == 0:
                        emit_uT(0, jns=[1])
                    if t < 4:
                        emit_w(0, t)
                continue
            emit_w(h, 0)
            # big/small alternation: a short ctx (few matmuls) never lands on
            # a psC slot whose stt was issued a mere one block earlier
            pend = [7, 0, 6, 1, 5, 2, 4, 3]
            last = h == HPC - 1
            # last head: also self-drain its own ctx blocks as soon as the
            # score tiles covering their key range have been exp'd, so the
            # final drain after tile4 is short
            self_after = {2: [0, 1], 3: [2, 3], 4: [4]} if last else {}
            fills = {1: pend[0:2], 2: pend[2:5], 3: pend[5:8]}
            for t in range(ntiles):
                emit_score_tile(h, t)
                if t == 0:
                    continue
                if t == 1:
                    # denominators of the previous head: cheap PE matmuls,
                    # one DVE copy, gates that head's stt chain
                    emit_D(h - 1, range(ST))
                    emit_Dcopy(h - 1, 0, ST)
                if t == 2:
                    emit_w(h, 1)
                    emit_w(h, 2)
                if t == 2 and last:
                    emit_D(h, [0, 1])
                    emit_Dcopy(h, 0, 2)
                if t == 3:
                    emit_w(h, 3)
                    if last:
                        emit_D(h, [2, 3, 4])
                        emit_Dcopy(h, 2, 5)
                for m_ in fills.get(t, ()):
                    emit_ctx(h - 1, m_)
                for m_ in self_after.get(t, ()):
                    emit_ctx(h, m_)
            if last:
                emit_D(h, [5, 6, 7])
                emit_Dcopy(h, 5, ST)
                for m_ in [7, 6, 5]:
                    emit_ctx(h, m_)

    nc.compile()   # bacc passes: split sync waits, move matmul waits to ldweights
    return nc


_nc_cache = {}


def _host_prep(x, attention_mask, Wq, bq, Wk, bk, Wv, bv, Wfc, bfc):
    """Host-side weight fusion (input-independent except x layout) ->
    (struct, nc, per-core in_maps, ybias)."""
    import ml_dtypes
    E4 = ml_dtypes.float8_e4m3

    x = np.asarray(x, np.float32)
    Wq64 = np.asarray(Wq, np.float64)
    Wk64 = np.asarray(Wk, np.float64)
    Wv64 = np.asarray(Wv, np.float64)
    Wfc64 = np.asarray(Wfc, np.float64)
    bq64 = np.asarray(bq, np.float64)
    bv64 = np.asarray(bv, np.float64)
    assert not bq64.any() and not np.asarray(bk, np.float64).any(), \
        "nonzero q/k bias not supported by this kernel variant"

    struct, ublocks = _mask_structure(attention_mask)
    key = struct[:3] + struct[4:]
    if key not in _nc_cache:
        _nc_cache[key] = _build(struct)
    nc = _nc_cache[key]

    scale = 1.0 / np.sqrt(np.float64(E))
    SH = 8.0      # w-hilo shift
    wa = np.empty((E, E * H), np.float64)
    wc = np.empty((E, E * H), np.float32)
    for g in range(H):
        gs = slice(g * E, (g + 1) * E)
        wa[:, gs] = scale * (Wk64[:, gs] @ Wq64[:, gs].T)
        wc[:, gs] = (Wv64[:, gs] @ Wfc64[gs, :]).astype(np.float32)
    wa8 = (wa * 256.0).astype(np.float32).astype(E4)
    # 3-term fp8 decomposition of w = x @ (64 C): see _build.emit_w
    chi8 = (wc * 64.0).astype(E4)
    chis8 = (wc * (64.0 / SH)).astype(E4)
    clos8 = ((wc * 64.0 - chi8.astype(np.float32)) * SH).astype(E4)
    ybias = (bv64 @ Wfc64 + np.asarray(bfc, np.float64)).astype(np.float32)

    # layout [identity, zero, mask blocks..., zero pad]: the mask matmul is a
    # DoubleRow pair (id, zero) x (block[uid], block[uid+1]) = 240*block[uid]
    em8 = np.concatenate(
        [240.0 * np.eye(P, dtype=np.float32)[None],
         np.zeros((1, P, P), np.float32),
         ublocks,
         np.zeros((1, P, P), np.float32)], axis=0).astype(E4)
    dcol = np.full((P, 2), 64.0, np.float32)   # psD rhs: w carries x64

    in_maps = []
    for c in range(NCORES):
        b, hg = divmod(c, 2)
        cs = slice(hg * EH, (hg + 1) * EH)
        xT = np.ascontiguousarray(x[b].T)
        xT8 = xT.astype(E4)
        in_maps.append({
            "xT8": xT8,
            "xhs8": (xT / SH).astype(E4),
            "xls8": ((xT - xT8.astype(np.float32)) * SH).astype(E4),
            "wa8": np.ascontiguousarray(wa8[:, cs]),
            "chi8": np.ascontiguousarray(chi8[:, cs]),
            "chis8": np.ascontiguousarray(chis8[:, cs]),
            "clos8": np.ascontiguousarray(clos8[:, cs]),
            "emask": em8,
            "dcol": dcol,
        })
    return key, nc, in_maps, ybias


def kernel(x, attention_mask, Wq, bq, Wk, bk, Wv, bv, Wfc, bfc, _trace=False):
    global LAST_RESULTS
    key, nc, in_maps, ybias = _host_prep(
        x, attention_mask, Wq, bq, Wk, bk, Wv, bv, Wfc, bfc)

    from concourse._compat import axon_active
    if axon_active() and not _trace:
        results = _run_pjrt_cached(key, nc, in_maps)
        LAST_RESULTS = None
    else:
        try:
            res = run_bass_kernel_spmd(nc, in_maps, core_ids=list(range(NCORES)),
                                       trace=_trace)
        except ModuleNotFoundError:
            # axon client without NTFF-profiling support: tracing disabled
            import os
            os.environ["BASS_NEVER_TRACE"] = "1"
            res = run_bass_kernel_spmd(nc, in_maps, core_ids=list(range(NCORES)),
                                       trace=False)
        LAST_RESULTS = res
        results = res.results
    out = np.empty((B, S, E), np.float32)
    for b in range(B):
        out[b] = results[2 * b]["y"] + results[2 * b + 1]["y"] + ybias
    return out


_jit_cache = {}


def _run_pjrt_cached(key, nc, in_maps):
    """bass2jax.run_bass_via_pjrt with the sharded jit cached per kernel
    structure, so repeated kernel() calls skip re-tracing (and with it the
    expensive NEFF recompile inside the neuronx_cc hook)."""
    import jax
    from jax.sharding import Mesh, PartitionSpec
    from jax.experimental.shard_map import shard_map
    from concourse import bass2jax
    import concourse.mybir as _mybir

    if key not in _jit_cache:
        bass2jax.install_neuronx_cc_hook()
        in_names, out_names, out_avals, zero_shapes = [], [], [], []
        for alloc in nc.m.functions[0].allocations:
            if not isinstance(alloc, _mybir.MemoryLocationSet):
                continue
            name = alloc.memorylocations[0].name
            if alloc.kind == "ExternalInput":
                if name != "partition_id":
                    in_names.append(name)
            elif alloc.kind == "ExternalOutput":
                shape = tuple(alloc.tensor_shape)
                dtype = _mybir.dt.np(alloc.dtype)
                out_names.append(name)
                out_avals.append(jax.core.ShapedArray(shape, dtype))
                zero_shapes.append((shape, dtype))
        n_params = len(in_names)
        n_outs = len(out_names)
        all_names = in_names + out_names + ["partition_id"]

        def _body(*args):
            operands = list(args)
            operands.append(bass2jax.partition_id_tensor())
            return tuple(bass2jax._bass_exec_p.bind(
                *operands,
                out_avals=tuple(out_avals),
                in_names=tuple(all_names),
                out_names=tuple(out_names),
                lowering_input_output_aliases=(),
                sim_require_finite=True,
                sim_require_nnan=True,
                nc=nc,
            ))

        devices = jax.devices()[:NCORES]
        mesh = Mesh(np.asarray(devices), ("core",))
        sharded = jax.jit(
            shard_map(_body, mesh=mesh,
                      in_specs=(PartitionSpec("core"),) * (n_params + n_outs),
                      out_specs=(PartitionSpec("core"),) * n_outs,
                      check_rep=False),
            donate_argnums=tuple(range(n_params, n_params + n_outs)),
            keep_unused=True,
        )
        _jit_cache[key] = (sharded, in_names, out_names, out_avals, zero_shapes)

    sharded, in_names, out_names, out_avals, zero_shapes = _jit_cache[key]
    concat_in = [
        np.concatenate([np.asarray(m[name]) for m in in_maps], axis=0)
        for name in in_names
    ]

    def _exec():
        concat_zeros = [np.zeros((NCORES * s[0], *s[1:]), d)
                        for s, d in zero_shapes]
        out_arrs = sharded(*concat_in, *concat_zeros)
        return [np.asarray(a) for a in out_arrs]

    try:
        out_arrs = _exec()
    except Exception:
        # transient device/transport flake: drop the failed call's effect
        # tokens (else jax's atexit block_until_ready re-raises even after a
        # successful retry) and retry once with fresh buffers
        try:
            from jax._src import dispatch as _jd
            _jd.runtime_tokens.clear()
        except Exception:
            pass
        out_arrs = _exec()
    return [
        {name: out_arrs[i].reshape(NCORES, *out_avals[i].shape)[c]
         for i, name in enumerate(out_names)}
        for c in range(NCORES)
    ]


# revision 53
# speedup vs baseline: 1.0029x; 1.0029x over previous
"""Trainium2 Bass kernel for a 12-head attention module (B=4, S=1024, E=256, H=12,
per-head dim = E — the module quirk that makes per-head weight fusion possible).

Sharding: 8 cores = 4 batches x 2 head-groups (6 heads each).  Each core computes
its partial fc projection; the host sums the two partials per batch element.

Algebraic fusion (host precomputes, in float64):
  A_h^T = scale * Wk_h @ Wq_h^T   (E x E)  ->  q/k projections collapse:
      uT_h    = A_h @ x^T            [E, S]
      scoresT = uT ki-block contracted with xT   [s_k, s_q]
  C_h = Wv_h @ Wfc_h   (E x E)  ->  the fc layer disappears:
      w_h  = x @ C_h                 [S, E]
      out  = sum_h softmax(scores_h) @ w_h
  bv and bfc become an exact host-side constant row:  out += bv @ Wfc + bfc.

Precision split (validated against the 2e-2 rel-l2 gate by host fp8 emulation):
  * uT and scores matmuls: fp8-e4m3 DoubleRow (two 128-contractions per
    instruction at 0.5 cycles/row = 4x the fp32r rate).  wa is host-scaled
    x256 into e4m3 range; exp applies scale=1/256.  Path error alone ~6e-3.
  * w = x @ (64 C): three fp8 DoubleRow terms  x_hi*C_hi + (x/8)*(8 dC) +
    (8 dx)*(8 C)  — hi/lo residual split on BOTH operands keeps w at
    ~fp32-precision (adds ~1e-4) at 0.75x the fp32r matmul cost.
  * probs @ w (ctx) stays fp32r: fp8 on either side of that product alone
    measures ~2.6e-2 — over the gate.  Total measured error: 5.8e-3.

The additive causal mask folds into the scores PSUM as fp8 DoubleRow matmuls
(lhsT = [240*I | 0] pair, rhs = mask-block pair, adding 240*em = 256*mask to
the 256-scaled scores), so masked entries reach exp() at score-225 and
underflow to exactly 0.0 in fp32 — no elementwise mask work on any engine.

Softmax denominators: tiny free-2 fp32r matmuls (probs-block^T @ 64-column)
accumulate 64*D per head into a 1-bank psD tile; one DVE copy + reciprocal
puts 1/(64 D) in SBUF (walrus allows the fused DVE op only one PSUM tensor
operand, and its ISA has no divide).  Per-head normalize + head-sum is then
ONE fused DVE op per q-block:
  scalar_tensor_tensor(acc, ctx_psum, recip_sbuf, acc, mult, add)

PSUM (8 banks): scores/uT tiles 2x[128,1024] double-buffered (4), w psum
[128,2,256] (1), ctx [128,256] double-buffered (2), psD (1).  PSUM->SBUF
copies may only run on DVE/Act (walrus: Pool cannot touch PSUM), so they are
balanced across both and software-pipelined: ctx+normalize of head h-1 and
the w matmuls of head h fill the PE while Act exps head h's score tiles.

Cost-model result: 53456 ns/core (PE 42.5us busy, Act 40.8, DVE 40.2), vs
75411 ns for the all-fp32r baseline = 1.41x.
"""

import numpy as np

import concourse.mybir as mybir
import concourse.tile as tile
from concourse import bacc
from concourse.bass_utils import run_bass_kernel_spmd

# Problem constants
B, S, E, H = 4, 1024, 256, 12
P = 128
NCORES = 8
HPC = H // 2            # heads per core
EH = E * HPC            # 1536 = per-core head width
KS_E = E // P           # 2 contraction subtiles over E
ST = S // P             # 8 row-blocks of S
EW = E                  # w width (denominators live in psD, not a ones col)
TILE_W = 1024           # scores psum tile width (2 PSUM banks)
BANK_W = 512            # fp32 elements per PSUM bank

FP8 = mybir.dt.float8e4
F32R = mybir.dt.float32r
DR = mybir.MatmulPerfMode.DoubleRow

LAST_RESULTS = None     # BassKernelResults of the most recent run (for harness)


def _mask_structure(attention_mask):
    """Classify 128x128 blocks of maskT -> (structure, unique fp8 mask blocks).

    Returns struct = (spans, tiles, covers, tot, nuniq) where
      spans[ki]  = (qa, qb) tight non-skip q-extent (block aligned) or None
      tiles      = per scores-psum-tile: (used_cols,
                     [(c0, c1, ki, q0), ...] bank-aligned matmul segments,
                     [(cm, uid), ...] mask-matmul blocks at tile col cm)
      covers[m]  = tuple of ki whose span fully covers q-block m
      tot        = packed probs width
    and ublocks = [nuniq, P, P] fp8-encodable f32 array of 4*maskT blocks.
    """
    m = np.asarray(attention_mask, dtype=np.float64).reshape(S, S)   # [q, k]
    mT = m.T                                                         # [k, q]

    uniq: dict[bytes, int] = {}
    blocks = {}
    for ki in range(ST):
        for qj in range(ST):
            blk = mT[ki * P:(ki + 1) * P, qj * P:(qj + 1) * P]
            if (blk <= -1000.0).all():
                blocks[(ki, qj)] = "skip"
            elif (blk == 0.0).all():
                blocks[(ki, qj)] = "one"
            else:
                # mask-matmul adds 240*em to the 256-scaled scores psum, so
                # em = mask*256/240 makes the exp see s+mask; fully-masked
                # entries clip to -240 -> scores-225 -> exp underflows to 0.0
                enc = np.clip(blk * (256.0 / 240.0), -240.0, 240.0).astype(np.float32)
                blocks[(ki, qj)] = uniq.setdefault(enc.tobytes(), len(uniq))

    allmask = np.full((P, P), -240.0, np.float32)
    spans = []
    for ki in range(ST):
        non = [qj for qj in range(ST) if blocks[(ki, qj)] != "skip"]
        spans.append(None if not non else (non[0] * P, (non[-1] + 1) * P))

    # packed probs layout: concatenated spans
    probs_off, tot = [], 0
    for ki in range(ST):
        probs_off.append(tot)
        if spans[ki] is not None:
            tot += spans[ki][1] - spans[ki][0]

    # mask-matmul blocks: every non-"one" block inside a span (interior skips
    # get the all--448 block so exp underflows to 0)
    mask_mm = {}    # packed block col -> uid
    for ki in range(ST):
        if spans[ki] is None:
            continue
        qa, qb = spans[ki]
        for qj in range(qa // P, qb // P):
            bl = blocks[(ki, qj)]
            if bl == "one":
                continue
            if bl == "skip":
                bl = uniq.setdefault(allmask.tobytes(), len(uniq))
            mask_mm[probs_off[ki] + qj * P - qa] = bl

    # scores psum tiles: split packed cols at TILE_W, segments at BANK_W
    ntiles = (tot + TILE_W - 1) // TILE_W
    tiles = []
    for t in range(ntiles):
        t0, t1 = t * TILE_W, min((t + 1) * TILE_W, tot)
        segs, masks = [], []
        for ki in range(ST):
            if spans[ki] is None:
                continue
            qa, qb = spans[ki]
            s0, s1 = probs_off[ki], probs_off[ki] + (qb - qa)
            a, b = max(s0, t0), min(s1, t1)
            pos = a
            while pos < b:
                nxt = min(b, (pos // BANK_W + 1) * BANK_W)
                segs.append((pos - t0, nxt - t0, ki, qa + pos - s0))
                pos = nxt
        for cm, uid in mask_mm.items():
            if t0 <= cm < t1:
                masks.append((cm - t0, uid))
        tiles.append((t1 - t0, tuple(segs), tuple(masks)))

    covers = []
    for m_ in range(ST):
        ks = tuple(ki for ki in range(ST)
                   if spans[ki] is not None
                   and spans[ki][0] <= m_ * P and spans[ki][1] >= (m_ + 1) * P)
        assert ks, (
            "attention row-block with no unmasked keys is not supported "
            "(reference softmax of an all-masked row is uniform)")
        covers.append(ks)

    nuniq = max(len(uniq), 1)
    ublocks = np.zeros((nuniq, P, P), np.float32)
    for key, uid in uniq.items():
        ublocks[uid] = np.frombuffer(key, np.float32).reshape(P, P)

    struct = (tuple(spans), tuple(tiles), tuple(covers),
              tuple(probs_off), tot, nuniq)
    return struct, ublocks


def _build(struct):
    spans, tiles, covers, probs_off, tot, nuniq = struct
    f32 = mybir.dt.float32
    Exp = mybir.ActivationFunctionType.Exp
    Alu = mybir.AluOpType

    nc = bacc.Bacc("TRN2")
    xT8_d = nc.dram_tensor("xT8", (E, S), FP8, kind="ExternalInput")
    xhs_d = nc.dram_tensor("xhs8", (E, S), FP8, kind="ExternalInput")
    xls_d = nc.dram_tensor("xls8", (E, S), FP8, kind="ExternalInput")
    wa8_d = nc.dram_tensor("wa8", (E, EH), FP8, kind="ExternalInput")
    chi_d = nc.dram_tensor("chi8", (E, EH), FP8, kind="ExternalInput")
    chis_d = nc.dram_tensor("chis8", (E, EH), FP8, kind="ExternalInput")
    clos_d = nc.dram_tensor("clos8", (E, EH), FP8, kind="ExternalInput")
    em_d = nc.dram_tensor("emask", (nuniq + 3, P, P), FP8, kind="ExternalInput")
    dcol_d = nc.dram_tensor("dcol", (P, 2), F32R, kind="ExternalInput")
    y_d = nc.dram_tensor("y", (S, E), f32, kind="ExternalOutput")

    ntiles = len(tiles)

    with tile.TileContext(nc) as tc, \
            tc.tile_pool(name="singles", bufs=1) as singles, \
            tc.tile_pool(name="heads", bufs=2) as heads, \
            tc.tile_pool(name="psA", bufs=2, space="PSUM") as psA, \
            tc.tile_pool(name="psW", bufs=1, space="PSUM") as psW, \
            tc.tile_pool(name="psC", bufs=2, space="PSUM") as psC, \
            tc.tile_pool(name="psD", bufs=1, space="PSUM") as psD:

        # ---- resident tensors, DMA'd in first-use order on 3 queues.
        # front-split so the first uT matmul's 128-col lhsT / 512-col rhs
        # unblock on tiny transfers instead of the full tensors ----
        h0 = slice(0, E)
        wa8_r = wa8_d[:, :].rearrange("(ko p) n -> p ko n", p=P)
        wa8_sb = singles.tile([P, KS_E, EH], FP8)
        nc.sync.dma_start(wa8_sb[:, :, 0:P], wa8_r[:, :, 0:P])
        xT8_sb = singles.tile([P, KS_E, S], FP8)
        xT8_r = xT8_d[:, :].rearrange("(ko p) n -> p ko n", p=P)
        nc.scalar.dma_start(xT8_sb[:, :, 0:BANK_W], xT8_r[:, :, 0:BANK_W])
        nc.sync.dma_start(wa8_sb[:, :, P:E], wa8_r[:, :, P:E])
        em_sb = singles.tile([P, nuniq + 3, P], FP8)
        nc.sync.dma_start(em_sb, em_d[:, :, :].rearrange("u p q -> p u q"))
        idz_sb = em_sb[:, 0:2, :]
        dcol_sb = singles.tile([P, 2], F32R)
        nc.sync.dma_start(dcol_sb, dcol_d[:, :])
        nc.scalar.dma_start(xT8_sb[:, :, BANK_W:S], xT8_r[:, :, BANK_W:S])  # Act idle at start
        chi_sb = singles.tile([P, KS_E, EH], FP8)
        chis_sb = singles.tile([P, KS_E, EH], FP8)
        clos_sb = singles.tile([P, KS_E, EH], FP8)
        for sb, d in ((chi_sb, chi_d), (chis_sb, chis_d), (clos_sb, clos_d)):
            nc.sync.dma_start(sb[:, :, h0],
                              d[:, h0].rearrange("(ko p) n -> p ko n", p=P))
        # x hi/lo shifts, front 256 cols first (w st0/st1 of head 0)
        xhs_sb = singles.tile([P, KS_E, S], FP8)
        xls_sb = singles.tile([P, KS_E, S], FP8)
        xhs_r = xhs_d[:, :].rearrange("(ko p) n -> p ko n", p=P)
        xls_r = xls_d[:, :].rearrange("(ko p) n -> p ko n", p=P)
        nc.gpsimd.dma_start(xhs_sb[:, :, 0:2 * P], xhs_r[:, :, 0:2 * P])
        nc.gpsimd.dma_start(xls_sb[:, :, 0:2 * P], xls_r[:, :, 0:2 * P])
        nc.gpsimd.dma_start(xhs_sb[:, :, 2 * P:S], xhs_r[:, :, 2 * P:S])
        nc.gpsimd.dma_start(xls_sb[:, :, 2 * P:S], xls_r[:, :, 2 * P:S])
        # heads 1..5 weights coalesced: one DMA per tensor, not per (h, tensor)
        hr = slice(E, EH)
        for sb, d in ((wa8_sb, wa8_d), (chi_sb, chi_d),
                      (chis_sb, chis_d), (clos_sb, clos_d)):
            nc.sync.dma_start(
                sb[:, :, hr], d[:, hr].rearrange("(ko p) n -> p ko n", p=P))
        acc_sb = singles.tile([P, ST, E], f32)
        nc.gpsimd.memset(acc_sb, 0.0)   # h==0 stt adds into zeroed acc

        state = {}   # per-head live tiles

        def emit_uT(h, jns=None):
            if jns is None:
                jns = range(S // BANK_W)
            if h in state and "uT" in state[h]:
                uT8 = state[h]["uT"]
            else:
                uT8 = heads.tile([P, KS_E, S], FP8, tag="uT", name="uT8")
                state[h] = {"uT": uT8}
            # psum tile jn holds [t0-jn | t1-jn] side by side, so each copy
            # moves one jn-half of BOTH t rows in a single strided instr; the
            # jn=0 copy lands first (early score tiles read low uT cols, and
            # at startup xT8's high half is still in flight)
            for jn in jns:
                js = slice(jn * BANK_W, (jn + 1) * BANK_W)
                ps = psA.tile([P, TILE_W], f32, tag="mm1024", name="ps_u")
                for t in range(KS_E):
                    nc.tensor.matmul(
                        ps[:, t * BANK_W:(t + 1) * BANK_W],
                        wa8_sb[:, :, h * E + t * P: h * E + (t + 1) * P],
                        xT8_sb[:, :, js],
                        start=True, stop=True, perf_mode=DR,
                    )
                pr = ps[:, :].rearrange("p (t n) -> p t n", t=KS_E)
                if jn == 0:
                    # tile0 only needs uT cols 0:128 (ki=0) — land them first;
                    # jn1 goes to Act in parallel (it idles at head boundary)
                    nc.vector.tensor_copy(uT8[:, :, 0:P], pr[:, :, 0:P])
                    nc.vector.tensor_copy(
                        uT8[:, :, P:BANK_W], pr[:, :, P:BANK_W])
                else:
                    nc.scalar.copy(uT8[:, :, js], pr)

        def emit_w(h, g2):
            # g2 in 0..3: two st-blocks per psW tile (single-bank, bufs=1)
            if g2 == 0:
                ww = heads.tile([P, ST, EW], F32R, tag="w", name="ww")
                state[h]["w"] = ww
            ww = state[h]["w"]
            he = slice(h * E, (h + 1) * E)
            psw = psW.tile([P, 2, E], f32, tag="psw", name="ps_w")
            for j in range(2):
                st = 2 * g2 + j
                ss = slice(st * P, (st + 1) * P)
                # w = x@(64 C) as 3 fp8 DoubleRow terms:
                #   x_hi*C_hi + (x/8)*(8 dC) + (8 dx)*(8C)
                for i, (xs, cs) in enumerate(
                        ((xT8_sb, chi_sb), (xhs_sb, clos_sb), (xls_sb, chis_sb))):
                    nc.tensor.matmul(
                        psw[:, j, :], xs[:, :, ss], cs[:, :, he],
                        start=(i == 0), stop=(i == 2), perf_mode=DR,
                    )
            # psum->sbuf copies may only run on DVE or Act (walrus forbids
            # Pool reading PSUM); mostly Act, some DVE, to balance engines
            if g2 >= 1:
                nc.vector.tensor_copy(ww[:, 2 * g2:2 * g2 + 2, :E], psw)
            else:
                nc.scalar.copy(ww[:, 2 * g2:2 * g2 + 2, :E], psw)

        def emit_probs(h):
            state[h]["probs"] = heads.tile([P, tot], F32R, tag="probs", name="probs")

        def emit_score_tile(h, t):
            used, segs, masks = tiles[t]
            uT8 = state[h]["uT"]
            probs = state[h]["probs"]
            ps = psA.tile([P, TILE_W], f32, tag="mm1024", name="ps_s")
            for (c0, c1, ki, q0) in segs:
                seg_masks = [(cm, uid) for cm, uid in masks if c0 <= cm < c1]
                nc.tensor.matmul(
                    ps[:, c0:c1],
                    uT8[:, :, ki * P:(ki + 1) * P],
                    xT8_sb[:, :, q0:q0 + (c1 - c0)],
                    start=True, stop=(not seg_masks), perf_mode=DR,
                )
                for i, (cm, uid) in enumerate(seg_masks):
                    nc.tensor.matmul(
                        ps[:, cm:cm + P], idz_sb, em_sb[:, 2 + uid:4 + uid, :],
                        start=False, stop=(i == len(seg_masks) - 1),
                        perf_mode=DR,
                    )
            nc.scalar.activation(probs[:, t * TILE_W: t * TILE_W + used],
                                 ps[:, :used], Exp, scale=1.0 / 256)

        def emit_D(h, ms):
            # softmax denominators D(q)*64 for q-blocks ms into this head's
            # psD tile: free-2 fp32r matmuls against the 64-valued column pair
            # (walrus allows stt only one PSUM tensor input, so D must reach
            # SBUF separately from the ctx psum)
            if "psD" not in state[h]:
                state[h]["psD"] = psD.tile([P, 2 * ST], f32, tag="psd",
                                           name="ps_d")
                state[h]["dsb"] = heads.tile([P, 2 * ST], f32, tag="dsb",
                                             name="dsb")
                state[h]["rsb"] = heads.tile([P, 2 * ST], f32, tag="rsb",
                                             name="rsb")
            psd = state[h]["psD"]
            probs = state[h]["probs"]
            for m_ in ms:
                ks_list = covers[m_]
                last = len(ks_list) - 1
                for idx, ki in enumerate(ks_list):
                    qa = spans[ki][0]
                    off = probs_off[ki]
                    nc.tensor.matmul(
                        psd[:, 2 * m_:2 * m_ + 2],
                        probs[:, off + m_ * P - qa: off + (m_ + 1) * P - qa],
                        dcol_sb,
                        start=(idx == 0), stop=(idx == last),
                    )

        def emit_Dcopy(h, lo, hi):
            # psD -> sbuf, then reciprocal (DVE stt has no divide in ISA)
            nc.vector.tensor_copy(state[h]["dsb"][:, 2 * lo:2 * hi],
                                  state[h]["psD"][:, 2 * lo:2 * hi])
            nc.vector.reciprocal(state[h]["rsb"][:, 2 * lo:2 * hi],
                                 state[h]["dsb"][:, 2 * lo:2 * hi])

        def emit_ctx(h, m_):
            probs = state[h]["probs"]
            ww = state[h]["w"]
            ks_list = covers[m_]
            ps = psC.tile([P, EW], f32, tag="ctx", name="ps_c")
            last = len(ks_list) - 1
            for idx, ki in enumerate(ks_list):
                qa = spans[ki][0]
                off = probs_off[ki]
                nc.tensor.matmul(
                    ps,
                    probs[:, off + m_ * P - qa: off + (m_ + 1) * P - qa],
                    ww[:, ki, :],
                    start=(idx == 0), stop=(idx == last),
                )
            # fused normalize + head accumulate: acc = ctx / D (+ acc).
            # h==0 uses op1=bypass; in1 must still be readable, so point it at
            # the psum (acc is uninitialized until h==0 writes it).
            rsb = state[h]["rsb"]
            nc.vector.scalar_tensor_tensor(
                acc_sb[:, m_, :], ps, rsb[:, 2 * m_:2 * m_ + 1],
                acc_sb[:, m_, :], Alu.mult, Alu.add)
            if h == HPC - 1:
                nc.sync.dma_start(y_d[m_ * P:(m_ + 1) * P, :], acc_sb[:, m_, :])

        # ---- software-pipelined schedule: ctx of head h-1 fills the gaps
        # while the Act engine exps head h's scores ----
        for h in range(HPC):
            emit_uT(h, jns=[0] if h == 0 else None)
            emit_probs(h)
            if h == 0:
                # startup: tiles 0-2 need only uT's jn0 half, so tile0 runs
                # before xT8's high half has even arrived; w groups fill gaps,
                # and head 0 self-drains its first ctx blocks (coverage-gated)
                for t in range(ntiles):
                    emit_score_tile(0, t)
                    if t == 0:
                        emit_uT(0, jns=[1])
                        emit_w(0, 0)
                    elif t == 1:
                        emit_w(0, 1)
                    elif t == 2:
                        emit_w(0, 2)
                        emit_D(0, [0, 1])
                        emit_Dcopy(0, 0, 2)
                        emit_ctx(0, 0)
                    elif t == 3:
                        emit_w(0, 3)
                        emit_D(0, [2, 3])
                        emit_Dcopy(0, 2, 4)
                        emit_ctx(0, 1)
                        emit_ctx(0, 2)
                emit_ctx(0, 3)
                continue
            emit_w(h, 0)
            # big/small alternation: a short ctx (few matmuls) never lands on
            # a psC slot whose stt was issued a mere one block earlier
            pend = [7, 4, 6, 5] if h == 1 else [7, 0, 6, 1, 5, 2, 4, 3]
            last = h == HPC - 1
            # last head: also self-drain its own ctx blocks as soon as the
            # score tiles covering their key range have been exp'd, so the
            # final drain after tile4 is short
            self_after = {2: [0, 1], 3: [2, 3], 4: [4]} if last else {}
            if h == 1:
                fills = {1: pend[0:2], 2: pend[2:3], 3: pend[3:4]}
            else:
                fills = {1: pend[0:2], 2: pend[2:5], 3: pend[5:8]}
            for t in range(ntiles):
                emit_score_tile(h, t)
                if t == 0:
                    continue
                if t == 1:
                    # denominators of the previous head: cheap PE matmuls,
                    # one DVE copy, gates that head's stt chain
                    emit_D(h - 1, range(ST))
                    emit_Dcopy(h - 1, 0, ST)
                if t == 2:
                    emit_w(h, 1)
                    emit_w(h, 2)
                if t == 2 and last:
                    emit_D(h, [0, 1])
                    emit_Dcopy(h, 0, 2)
                if t == 3:
                    emit_w(h, 3)
                    if last:
                        emit_D(h, [2, 3, 4])
                        emit_Dcopy(h, 2, 5)
                for m_ in fills.get(t, ()):
                    emit_ctx(h - 1, m_)
                for m_ in self_after.get(t, ()):
                    emit_ctx(h, m_)
            if last:
                emit_D(h, [5, 6, 7])
                emit_Dcopy(h, 5, ST)
                for m_ in [7, 6, 5]:
                    emit_ctx(h, m_)

    nc.compile()   # bacc passes: split sync waits, move matmul waits to ldweights
    return nc


_nc_cache = {}


def _host_prep(x, attention_mask, Wq, bq, Wk, bk, Wv, bv, Wfc, bfc):
    """Host-side weight fusion (input-independent except x layout) ->
    (struct, nc, per-core in_maps, ybias)."""
    import ml_dtypes
    E4 = ml_dtypes.float8_e4m3

    x = np.asarray(x, np.float32)
    Wq64 = np.asarray(Wq, np.float64)
    Wk64 = np.asarray(Wk, np.float64)
    Wv64 = np.asarray(Wv, np.float64)
    Wfc64 = np.asarray(Wfc, np.float64)
    bq64 = np.asarray(bq, np.float64)
    bv64 = np.asarray(bv, np.float64)
    assert not bq64.any() and not np.asarray(bk, np.float64).any(), \
        "nonzero q/k bias not supported by this kernel variant"

    struct, ublocks = _mask_structure(attention_mask)
    key = struct[:3] + struct[4:]
    if key not in _nc_cache:
        _nc_cache[key] = _build(struct)
    nc = _nc_cache[key]

    scale = 1.0 / np.sqrt(np.float64(E))
    SH = 8.0      # w-hilo shift
    wa = np.empty((E, E * H), np.float64)
    wc = np.empty((E, E * H), np.float32)
    for g in range(H):
        gs = slice(g * E, (g + 1) * E)
        wa[:, gs] = scale * (Wk64[:, gs] @ Wq64[:, gs].T)
        wc[:, gs] = (Wv64[:, gs] @ Wfc64[gs, :]).astype(np.float32)
    wa8 = (wa * 256.0).astype(np.float32).astype(E4)
    # 3-term fp8 decomposition of w = x @ (64 C): see _build.emit_w
    chi8 = (wc * 64.0).astype(E4)
    chis8 = (wc * (64.0 / SH)).astype(E4)
    clos8 = ((wc * 64.0 - chi8.astype(np.float32)) * SH).astype(E4)
    ybias = (bv64 @ Wfc64 + np.asarray(bfc, np.float64)).astype(np.float32)

    # layout [identity, zero, mask blocks..., zero pad]: the mask matmul is a
    # DoubleRow pair (id, zero) x (block[uid], block[uid+1]) = 240*block[uid]
    em8 = np.concatenate(
        [240.0 * np.eye(P, dtype=np.float32)[None],
         np.zeros((1, P, P), np.float32),
         ublocks,
         np.zeros((1, P, P), np.float32)], axis=0).astype(E4)
    dcol = np.full((P, 2), 64.0, np.float32)   # psD rhs: w carries x64

    in_maps = []
    for c in range(NCORES):
        b, hg = divmod(c, 2)
        cs = slice(hg * EH, (hg + 1) * EH)
        xT = np.ascontiguousarray(x[b].T)
        xT8 = xT.astype(E4)
        in_maps.append({
            "xT8": xT8,
            "xhs8": (xT / SH).astype(E4),
            "xls8": ((xT - xT8.astype(np.float32)) * SH).astype(E4),
            "wa8": np.ascontiguousarray(wa8[:, cs]),
            "chi8": np.ascontiguousarray(chi8[:, cs]),
            "chis8": np.ascontiguousarray(chis8[:, cs]),
            "clos8": np.ascontiguousarray(clos8[:, cs]),
            "emask": em8,
            "dcol": dcol,
        })
    return key, nc, in_maps, ybias


def kernel(x, attention_mask, Wq, bq, Wk, bk, Wv, bv, Wfc, bfc, _trace=False):
    global LAST_RESULTS
    key, nc, in_maps, ybias = _host_prep(
        x, attention_mask, Wq, bq, Wk, bk, Wv, bv, Wfc, bfc)

    from concourse._compat import axon_active
    if axon_active() and not _trace:
        results = _run_pjrt_cached(key, nc, in_maps)
        LAST_RESULTS = None
    else:
        try:
            res = run_bass_kernel_spmd(nc, in_maps, core_ids=list(range(NCORES)),
                                       trace=_trace)
        except ModuleNotFoundError:
            # axon client without NTFF-profiling support: tracing disabled
            import os
            os.environ["BASS_NEVER_TRACE"] = "1"
            res = run_bass_kernel_spmd(nc, in_maps, core_ids=list(range(NCORES)),
                                       trace=False)
        LAST_RESULTS = res
        results = res.results
    out = np.empty((B, S, E), np.float32)
    for b in range(B):
        out[b] = results[2 * b]["y"] + results[2 * b + 1]["y"] + ybias
    return out


_jit_cache = {}


def _run_pjrt_cached(key, nc, in_maps):
    """bass2jax.run_bass_via_pjrt with the sharded jit cached per kernel
    structure, so repeated kernel() calls skip re-tracing (and with it the
    expensive NEFF recompile inside the neuronx_cc hook)."""
    import jax
    from jax.sharding import Mesh, PartitionSpec
    from jax.experimental.shard_map import shard_map
    from concourse import bass2jax
    import concourse.mybir as _mybir

    if key not in _jit_cache:
        bass2jax.install_neuronx_cc_hook()
        in_names, out_names, out_avals, zero_shapes = [], [], [], []
        for alloc in nc.m.functions[0].allocations:
            if not isinstance(alloc, _mybir.MemoryLocationSet):
                continue
            name = alloc.memorylocations[0].name
            if alloc.kind == "ExternalInput":
                if name != "partition_id":
                    in_names.append(name)
            elif alloc.kind == "ExternalOutput":
                shape = tuple(alloc.tensor_shape)
                dtype = _mybir.dt.np(alloc.dtype)
                out_names.append(name)
                out_avals.append(jax.core.ShapedArray(shape, dtype))
                zero_shapes.append((shape, dtype))
        n_params = len(in_names)
        n_outs = len(out_names)
        all_names = in_names + out_names + ["partition_id"]

        def _body(*args):
            operands = list(args)
            operands.append(bass2jax.partition_id_tensor())
            return tuple(bass2jax._bass_exec_p.bind(
                *operands,
                out_avals=tuple(out_avals),
                in_names=tuple(all_names),
                out_names=tuple(out_names),
                lowering_input_output_aliases=(),
                sim_require_finite=True,
                sim_require_nnan=True,
                nc=nc,
            ))

        devices = jax.devices()[:NCORES]
        mesh = Mesh(np.asarray(devices), ("core",))
        sharded = jax.jit(
            shard_map(_body, mesh=mesh,
                      in_specs=(PartitionSpec("core"),) * (n_params + n_outs),
                      out_specs=(PartitionSpec("core"),) * n_outs,
                      check_rep=False),
            donate_argnums=tuple(range(n_params, n_params + n_outs)),
            keep_unused=True,
        )
        _jit_cache[key] = (sharded, in_names, out_names, out_avals, zero_shapes)

    sharded, in_names, out_names, out_avals, zero_shapes = _jit_cache[key]
    concat_in = [
        np.concatenate([np.asarray(m[name]) for m in in_maps], axis=0)
        for name in in_names
    ]

    def _exec():
        concat_zeros = [np.zeros((NCORES * s[0], *s[1:]), d)
                        for s, d in zero_shapes]
        out_arrs = sharded(*concat_in, *concat_zeros)
        return [np.asarray(a) for a in out_arrs]

    try:
        out_arrs = _exec()
    except Exception:
        # transient device/transport flake: drop the failed call's effect
        # tokens (else jax's atexit block_until_ready re-raises even after a
        # successful retry) and retry once with fresh buffers
        try:
            from jax._src import dispatch as _jd
            _jd.runtime_tokens.clear()
        except Exception:
            pass
        out_arrs = _exec()
    return [
        {name: out_arrs[i].reshape(NCORES, *out_avals[i].shape)[c]
         for i, name in enumerate(out_names)}
        for c in range(NCORES)
    ]


# revision 61
# speedup vs baseline: 1.0043x; 1.0015x over previous
"""Trainium2 Bass kernel for a 12-head attention module (B=4, S=1024, E=256, H=12,
per-head dim = E — the module quirk that makes per-head weight fusion possible).

Sharding: 8 cores = 4 batches x 2 head-groups (6 heads each).  Each core computes
its partial fc projection; the host sums the two partials per batch element.

Algebraic fusion (host precomputes, in float64):
  A_h^T = scale * Wk_h @ Wq_h^T   (E x E)  ->  q/k projections collapse:
      uT_h    = A_h @ x^T            [E, S]
      scoresT = uT ki-block contracted with xT   [s_k, s_q]
  C_h = Wv_h @ Wfc_h   (E x E)  ->  the fc layer disappears:
      w_h  = x @ C_h                 [S, E]
      out  = sum_h softmax(scores_h) @ w_h
  bv and bfc become an exact host-side constant row:  out += bv @ Wfc + bfc.

Precision split (validated against the 2e-2 rel-l2 gate by host fp8 emulation):
  * uT and scores matmuls: fp8-e4m3 DoubleRow (two 128-contractions per
    instruction at 0.5 cycles/row = 4x the fp32r rate).  wa is host-scaled
    x256 into e4m3 range; exp applies scale=1/256.  Path error alone ~6e-3.
  * w = x @ (64 C): three fp8 DoubleRow terms  x_hi*C_hi + (x/8)*(8 dC) +
    (8 dx)*(8 C)  — hi/lo residual split on BOTH operands keeps w at
    ~fp32-precision (adds ~1e-4) at 0.75x the fp32r matmul cost.
  * probs @ w (ctx) stays fp32r: fp8 on either side of that product alone
    measures ~2.6e-2 — over the gate.  Total measured error: 5.8e-3.

The additive causal mask folds into the scores PSUM as fp8 DoubleRow matmuls
(lhsT = [240*I | 0] pair, rhs = mask-block pair, adding 240*em = 256*mask to
the 256-scaled scores), so masked entries reach exp() at score-225 and
underflow to exactly 0.0 in fp32 — no elementwise mask work on any engine.

Softmax denominators: tiny free-2 fp32r matmuls (probs-block^T @ 64-column)
accumulate 64*D per head into a 1-bank psD tile; one DVE copy + reciprocal
puts 1/(64 D) in SBUF (walrus allows the fused DVE op only one PSUM tensor
operand, and its ISA has no divide).  Per-head normalize + head-sum is then
ONE fused DVE op per q-block:
  scalar_tensor_tensor(acc, ctx_psum, recip_sbuf, acc, mult, add)

PSUM (8 banks): scores/uT tiles 2x[128,1024] double-buffered (4), w psum
[128,2,256] (1), ctx [128,256] double-buffered (2), psD (1).  PSUM->SBUF
copies may only run on DVE/Act (walrus: Pool cannot touch PSUM), so they are
balanced across both and software-pipelined: ctx+normalize of head h-1 and
the w matmuls of head h fill the PE while Act exps head h's score tiles.

Cost-model result: 53226 ns/core (PE 41.3us busy, Act 40.8, DVE 40.6), vs
75411 ns for the all-fp32r baseline = 1.41x.
"""

import numpy as np

import concourse.mybir as mybir
import concourse.tile as tile
from concourse import bacc
from concourse.bass_utils import run_bass_kernel_spmd

# Problem constants
B, S, E, H = 4, 1024, 256, 12
P = 128
NCORES = 8
HPC = H // 2            # heads per core
EH = E * HPC            # 1536 = per-core head width
KS_E = E // P           # 2 contraction subtiles over E
ST = S // P             # 8 row-blocks of S
EW = E                  # w width (denominators live in psD, not a ones col)
TILE_W = 1024           # scores psum tile width (2 PSUM banks)
BANK_W = 512            # fp32 elements per PSUM bank

FP8 = mybir.dt.float8e4
F32R = mybir.dt.float32r
DR = mybir.MatmulPerfMode.DoubleRow

LAST_RESULTS = None     # BassKernelResults of the most recent run (for harness)


def _mask_structure(attention_mask):
    """Classify 128x128 blocks of maskT -> (structure, unique fp8 mask blocks).

    Returns struct = (spans, tiles, covers, tot, nuniq) where
      spans[ki]  = (qa, qb) tight non-skip q-extent (block aligned) or None
      tiles      = per scores-psum-tile: (used_cols,
                     [(c0, c1, ki, q0), ...] bank-aligned matmul segments,
                     [(cm, uid), ...] mask-matmul blocks at tile col cm)
      covers[m]  = tuple of ki whose span fully covers q-block m
      tot        = packed probs width
    and ublocks = [nuniq, P, P] fp8-encodable f32 array of 4*maskT blocks.
    """
    m = np.asarray(attention_mask, dtype=np.float64).reshape(S, S)   # [q, k]
    mT = m.T                                                         # [k, q]

    uniq: dict[bytes, int] = {}
    blocks = {}
    for ki in range(ST):
        for qj in range(ST):
            blk = mT[ki * P:(ki + 1) * P, qj * P:(qj + 1) * P]
            if (blk <= -1000.0).all():
                blocks[(ki, qj)] = "skip"
            elif (blk == 0.0).all():
                blocks[(ki, qj)] = "one"
            else:
                # mask-matmul adds 240*em to the 256-scaled scores psum, so
                # em = mask*256/240 makes the exp see s+mask; fully-masked
                # entries clip to -240 -> scores-225 -> exp underflows to 0.0
                enc = np.clip(blk * (256.0 / 240.0), -240.0, 240.0).astype(np.float32)
                blocks[(ki, qj)] = uniq.setdefault(enc.tobytes(), len(uniq))

    allmask = np.full((P, P), -240.0, np.float32)
    spans = []
    for ki in range(ST):
        non = [qj for qj in range(ST) if blocks[(ki, qj)] != "skip"]
        spans.append(None if not non else (non[0] * P, (non[-1] + 1) * P))

    # packed probs layout: concatenated spans
    probs_off, tot = [], 0
    for ki in range(ST):
        probs_off.append(tot)
        if spans[ki] is not None:
            tot += spans[ki][1] - spans[ki][0]

    # mask-matmul blocks: every non-"one" block inside a span (interior skips
    # get the all--240 block so exp underflows to 0)
    mask_mm = {}    # packed block col -> uid
    for ki in range(ST):
        if spans[ki] is None:
            continue
        qa, qb = spans[ki]
        for qj in range(qa // P, qb // P):
            bl = blocks[(ki, qj)]
            if bl == "one":
                continue
            if bl == "skip":
                bl = uniq.setdefault(allmask.tobytes(), len(uniq))
            mask_mm[probs_off[ki] + qj * P - qa] = bl

    # scores psum tiles: split packed cols at TILE_W, segments at BANK_W
    ntiles = (tot + TILE_W - 1) // TILE_W
    tiles = []
    for t in range(ntiles):
        t0, t1 = t * TILE_W, min((t + 1) * TILE_W, tot)
        segs, masks = [], []
        for ki in range(ST):
            if spans[ki] is None:
                continue
            qa, qb = spans[ki]
            s0, s1 = probs_off[ki], probs_off[ki] + (qb - qa)
            a, b = max(s0, t0), min(s1, t1)
            pos = a
            while pos < b:
                nxt = min(b, (pos // BANK_W + 1) * BANK_W)
                segs.append((pos - t0, nxt - t0, ki, qa + pos - s0))
                pos = nxt
        for cm, uid in mask_mm.items():
            if t0 <= cm < t1:
                masks.append((cm - t0, uid))
        tiles.append((t1 - t0, tuple(segs), tuple(masks)))

    covers = []
    for m_ in range(ST):
        ks = tuple(ki for ki in range(ST)
                   if spans[ki] is not None
                   and spans[ki][0] <= m_ * P and spans[ki][1] >= (m_ + 1) * P)
        assert ks, (
            "attention row-block with no unmasked keys is not supported "
            "(reference softmax of an all-masked row is uniform)")
        covers.append(ks)

    nuniq = max(len(uniq), 1)
    ublocks = np.zeros((nuniq, P, P), np.float32)
    for key, uid in uniq.items():
        ublocks[uid] = np.frombuffer(key, np.float32).reshape(P, P)

    struct = (tuple(spans), tuple(tiles), tuple(covers),
              tuple(probs_off), tot, nuniq)
    return struct, ublocks


def _build(struct):
    spans, tiles, covers, probs_off, tot, nuniq = struct
    f32 = mybir.dt.float32
    Exp = mybir.ActivationFunctionType.Exp
    Alu = mybir.AluOpType

    nc = bacc.Bacc("TRN2")
    xT8_d = nc.dram_tensor("xT8", (E, S), FP8, kind="ExternalInput")
    xhs_d = nc.dram_tensor("xhs8", (E, S), FP8, kind="ExternalInput")
    xls_d = nc.dram_tensor("xls8", (E, S), FP8, kind="ExternalInput")
    wa8_d = nc.dram_tensor("wa8", (E, EH), FP8, kind="ExternalInput")
    chi_d = nc.dram_tensor("chi8", (E, EH), FP8, kind="ExternalInput")
    chis_d = nc.dram_tensor("chis8", (E, EH), FP8, kind="ExternalInput")
    clos_d = nc.dram_tensor("clos8", (E, EH), FP8, kind="ExternalInput")
    em_d = nc.dram_tensor("emask", (nuniq + 3, P, P), FP8, kind="ExternalInput")
    dcol_d = nc.dram_tensor("dcol", (P, 2), F32R, kind="ExternalInput")
    y_d = nc.dram_tensor("y", (S, E), f32, kind="ExternalOutput")

    ntiles = len(tiles)

    with tile.TileContext(nc) as tc, \
            tc.tile_pool(name="singles", bufs=1) as singles, \
            tc.tile_pool(name="heads", bufs=2) as heads, \
            tc.tile_pool(name="psA", bufs=2, space="PSUM") as psA, \
            tc.tile_pool(name="psW", bufs=1, space="PSUM") as psW, \
            tc.tile_pool(name="psC", bufs=2, space="PSUM") as psC, \
            tc.tile_pool(name="psD", bufs=1, space="PSUM") as psD:

        # ---- resident tensors, DMA'd in first-use order on 3 queues.
        # front-split so the first uT matmul's 128-col lhsT / 512-col rhs
        # unblock on tiny transfers instead of the full tensors ----
        h0 = slice(0, E)
        wa8_r = wa8_d[:, :].rearrange("(ko p) n -> p ko n", p=P)
        wa8_sb = singles.tile([P, KS_E, EH], FP8)
        nc.sync.dma_start(wa8_sb[:, :, 0:P], wa8_r[:, :, 0:P])
        xT8_sb = singles.tile([P, KS_E, S], FP8)
        xT8_r = xT8_d[:, :].rearrange("(ko p) n -> p ko n", p=P)
        nc.scalar.dma_start(xT8_sb[:, :, 0:BANK_W], xT8_r[:, :, 0:BANK_W])
        nc.sync.dma_start(wa8_sb[:, :, P:E], wa8_r[:, :, P:E])
        em_sb = singles.tile([P, nuniq + 3, P], FP8)
        nc.sync.dma_start(em_sb, em_d[:, :, :].rearrange("u p q -> p u q"))
        idz_sb = em_sb[:, 0:2, :]
        dcol_sb = singles.tile([P, 2], F32R)
        nc.sync.dma_start(dcol_sb, dcol_d[:, :])
        nc.scalar.dma_start(xT8_sb[:, :, BANK_W:S], xT8_r[:, :, BANK_W:S])  # Act idle at start
        chi_sb = singles.tile([P, KS_E, EH], FP8)
        chis_sb = singles.tile([P, KS_E, EH], FP8)
        clos_sb = singles.tile([P, KS_E, EH], FP8)
        # head-0 C tensors split across both hwdge queues so w(0) isn't
        # stuck behind a serial trigger chain
        nc.sync.dma_start(chi_sb[:, :, h0],
                          chi_d[:, h0].rearrange("(ko p) n -> p ko n", p=P))
        for sb, d in ((chis_sb, chis_d), (clos_sb, clos_d)):
            nc.scalar.dma_start(sb[:, :, h0],
                                d[:, h0].rearrange("(ko p) n -> p ko n", p=P))
        # x hi/lo shifts, front 256 cols first (w st0/st1 of head 0)
        xhs_sb = singles.tile([P, KS_E, S], FP8)
        xls_sb = singles.tile([P, KS_E, S], FP8)
        xhs_r = xhs_d[:, :].rearrange("(ko p) n -> p ko n", p=P)
        xls_r = xls_d[:, :].rearrange("(ko p) n -> p ko n", p=P)
        nc.gpsimd.dma_start(xhs_sb[:, :, 0:2 * P], xhs_r[:, :, 0:2 * P])
        nc.gpsimd.dma_start(xls_sb[:, :, 0:2 * P], xls_r[:, :, 0:2 * P])
        nc.gpsimd.dma_start(xhs_sb[:, :, 2 * P:S], xhs_r[:, :, 2 * P:S])
        nc.gpsimd.dma_start(xls_sb[:, :, 2 * P:S], xls_r[:, :, 2 * P:S])
        # heads 1..5 weights coalesced: one DMA per tensor, not per (h, tensor)
        hr = slice(E, EH)
        for sb, d in ((wa8_sb, wa8_d), (chi_sb, chi_d),
                      (chis_sb, chis_d), (clos_sb, clos_d)):
            nc.sync.dma_start(
                sb[:, :, hr], d[:, hr].rearrange("(ko p) n -> p ko n", p=P))
        acc_sb = singles.tile([P, ST, E], f32)
        nc.gpsimd.memset(acc_sb, 0.0)   # h==0 stt adds into zeroed acc

        state = {}   # per-head live tiles

        def emit_uT(h, jns=None):
            if jns is None:
                jns = range(S // BANK_W)
            if h in state and "uT" in state[h]:
                uT8 = state[h]["uT"]
            else:
                uT8 = heads.tile([P, KS_E, S], FP8, tag="uT", name="uT8")
                state[h] = {"uT": uT8}
            # psum tile jn holds [t0-jn | t1-jn] side by side, so each copy
            # moves one jn-half of BOTH t rows in a single strided instr; the
            # jn=0 copy lands first (early score tiles read low uT cols, and
            # at startup xT8's high half is still in flight)
            for jn in jns:
                js = slice(jn * BANK_W, (jn + 1) * BANK_W)
                ps = psA.tile([P, TILE_W], f32, tag="mm1024", name="ps_u")
                for t in range(KS_E):
                    nc.tensor.matmul(
                        ps[:, t * BANK_W:(t + 1) * BANK_W],
                        wa8_sb[:, :, h * E + t * P: h * E + (t + 1) * P],
                        xT8_sb[:, :, js],
                        start=True, stop=True, perf_mode=DR,
                    )
                pr = ps[:, :].rearrange("p (t n) -> p t n", t=KS_E)
                if jn == 0:
                    # tile0 only needs uT cols 0:128 (ki=0) — land them first;
                    # jn1 goes to Act in parallel (it idles at head boundary)
                    nc.vector.tensor_copy(uT8[:, :, 0:P], pr[:, :, 0:P])
                    nc.vector.tensor_copy(
                        uT8[:, :, P:BANK_W], pr[:, :, P:BANK_W])
                else:
                    nc.scalar.copy(uT8[:, :, js], pr)

        def emit_w(h, g2):
            # g2 in 0..3: two st-blocks per psW tile (single-bank, bufs=1)
            if g2 == 0:
                ww = heads.tile([P, ST, EW], F32R, tag="w", name="ww")
                state[h]["w"] = ww
            ww = state[h]["w"]
            he = slice(h * E, (h + 1) * E)
            psw = psW.tile([P, 2, E], f32, tag="psw", name="ps_w")
            for j in range(2):
                st = 2 * g2 + j
                ss = slice(st * P, (st + 1) * P)
                # w = x@(64 C) as 3 fp8 DoubleRow terms:
                #   x_hi*C_hi + (x/8)*(8 dC) + (8 dx)*(8C)
                for i, (xs, cs) in enumerate(
                        ((xT8_sb, chi_sb), (xhs_sb, clos_sb), (xls_sb, chis_sb))):
                    nc.tensor.matmul(
                        psw[:, j, :], xs[:, :, ss], cs[:, :, he],
                        start=(i == 0), stop=(i == 2), perf_mode=DR,
                    )
            # psum->sbuf copies may only run on DVE or Act (walrus forbids
            # Pool reading PSUM); mostly Act, some DVE, to balance engines
            if g2 >= 1:
                nc.vector.tensor_copy(ww[:, 2 * g2:2 * g2 + 2, :E], psw)
            else:
                nc.scalar.copy(ww[:, 2 * g2:2 * g2 + 2, :E], psw)

        def emit_probs(h):
            state[h]["probs"] = heads.tile([P, tot], F32R, tag="probs", name="probs")

        def emit_score_tile(h, t):
            used, segs, masks = tiles[t]
            uT8 = state[h]["uT"]
            probs = state[h]["probs"]
            ps = psA.tile([P, TILE_W], f32, tag="mm1024", name="ps_s")
            for (c0, c1, ki, q0) in segs:
                seg_masks = [(cm, uid) for cm, uid in masks if c0 <= cm < c1]
                nc.tensor.matmul(
                    ps[:, c0:c1],
                    uT8[:, :, ki * P:(ki + 1) * P],
                    xT8_sb[:, :, q0:q0 + (c1 - c0)],
                    start=True, stop=(not seg_masks), perf_mode=DR,
                )
                for i, (cm, uid) in enumerate(seg_masks):
                    nc.tensor.matmul(
                        ps[:, cm:cm + P], idz_sb, em_sb[:, 2 + uid:4 + uid, :],
                        start=False, stop=(i == len(seg_masks) - 1),
                        perf_mode=DR,
                    )
            nc.scalar.activation(probs[:, t * TILE_W: t * TILE_W + used],
                                 ps[:, :used], Exp, scale=1.0 / 256)

        def emit_D(h, ms):
            # softmax denominators D(q)*64 for q-blocks ms into this head's
            # psD tile: free-2 fp32r matmuls against the 64-valued column pair
            # (walrus allows stt only one PSUM tensor input, so D must reach
            # SBUF separately from the ctx psum)
            if "psD" not in state[h]:
                state[h]["psD"] = psD.tile([P, 2 * ST], f32, tag="psd",
                                           name="ps_d")
                state[h]["dsb"] = heads.tile([P, 2 * ST], f32, tag="dsb",
                                             name="dsb")
                state[h]["rsb"] = heads.tile([P, 2 * ST], f32, tag="rsb",
                                             name="rsb")
            psd = state[h]["psD"]
            probs = state[h]["probs"]
            for m_ in ms:
                ks_list = covers[m_]
                last = len(ks_list) - 1
                for idx, ki in enumerate(ks_list):
                    qa = spans[ki][0]
                    off = probs_off[ki]
                    nc.tensor.matmul(
                        psd[:, 2 * m_:2 * m_ + 2],
                        probs[:, off + m_ * P - qa: off + (m_ + 1) * P - qa],
                        dcol_sb,
                        start=(idx == 0), stop=(idx == last),
                    )

        def emit_Dcopy(h, lo, hi):
            # psD -> sbuf, then reciprocal (DVE stt has no divide in ISA)
            nc.vector.tensor_copy(state[h]["dsb"][:, 2 * lo:2 * hi],
                                  state[h]["psD"][:, 2 * lo:2 * hi])
            nc.vector.reciprocal(state[h]["rsb"][:, 2 * lo:2 * hi],
                                 state[h]["dsb"][:, 2 * lo:2 * hi])

        def emit_ctx(h, m_):
            probs = state[h]["probs"]
            ww = state[h]["w"]
            ks_list = covers[m_]
            ps = psC.tile([P, EW], f32, tag="ctx", name="ps_c")
            last = len(ks_list) - 1
            for idx, ki in enumerate(ks_list):
                qa = spans[ki][0]
                off = probs_off[ki]
                nc.tensor.matmul(
                    ps,
                    probs[:, off + m_ * P - qa: off + (m_ + 1) * P - qa],
                    ww[:, ki, :],
                    start=(idx == 0), stop=(idx == last),
                )
            # fused normalize + head accumulate: acc += ctx * (1/(64 D));
            # acc starts memset to zero so every head can just add
            rsb = state[h]["rsb"]
            nc.vector.scalar_tensor_tensor(
                acc_sb[:, m_, :], ps, rsb[:, 2 * m_:2 * m_ + 1],
                acc_sb[:, m_, :], Alu.mult, Alu.add)
            if h == HPC - 1:
                nc.sync.dma_start(y_d[m_ * P:(m_ + 1) * P, :], acc_sb[:, m_, :])

        # ---- software-pipelined schedule: ctx of head h-1 fills the gaps
        # while the Act engine exps head h's scores ----
        for h in range(HPC):
            emit_uT(h, jns=[0] if h == 0 else None)
            emit_probs(h)
            if h == 0:
                # startup: tiles 0-2 need only uT's jn0 half, so tile0 runs
                # before xT8's high half has even arrived; w groups fill gaps,
                # and head 0 self-drains its first ctx blocks (coverage-gated)
                for t in range(ntiles):
                    emit_score_tile(0, t)
                    if t == 0:
                        emit_uT(0, jns=[1])
                        emit_w(0, 0)
                    elif t == 1:
                        emit_w(0, 1)
                    elif t == 2:
                        emit_w(0, 2)
                        emit_D(0, [0, 1])
                        emit_Dcopy(0, 0, 2)
                        emit_ctx(0, 0)
                    elif t == 3:
                        emit_w(0, 3)
                        emit_D(0, [2, 3])
                        emit_Dcopy(0, 2, 4)
                        emit_ctx(0, 1)
                        emit_ctx(0, 2)
                emit_ctx(0, 3)
                continue
            emit_w(h, 0)
            # big/small alternation: a short ctx (few matmuls) never lands on
            # a psC slot whose stt was issued a mere one block earlier
            pend = [7, 4, 6, 5] if h == 1 else [7, 0, 6, 1, 5, 2, 4, 3]
            last = h == HPC - 1
            # last head: also self-drain its own ctx blocks as soon as the
            # score tiles covering their key range have been exp'd, so the
            # final drain after tile4 is short
            self_after = {2: [0], 3: [1, 2], 4: [3, 4]} if last else {}
            if h == 1:
                fills = {1: pend[0:2], 2: pend[2:3], 3: pend[3:4]}
            else:
                fills = {1: pend[0:2], 2: pend[2:5], 3: pend[5:8]}
            for t in range(ntiles):
                emit_score_tile(h, t)
                if t == 0:
                    continue
                if t == 1:
                    # denominators of the previous head: cheap PE matmuls,
                    # one DVE copy, gates that head's stt chain
                    emit_D(h - 1, range(ST))
                    emit_Dcopy(h - 1, 0, ST)
                if t == 2:
                    emit_w(h, 1)
                    emit_w(h, 2)
                if t == 2 and last:
                    emit_D(h, [0, 1])
                    emit_Dcopy(h, 0, 2)
                if t == 3:
                    emit_w(h, 3)
                    if last:
                        emit_D(h, [2, 3, 4])
                        emit_Dcopy(h, 2, 5)
                for m_ in fills.get(t, ()):
                    emit_ctx(h - 1, m_)
                for m_ in self_after.get(t, ()):
                    emit_ctx(h, m_)
            if last:
                emit_D(h, [5, 6, 7])
                emit_Dcopy(h, 5, ST)
                for m_ in [7, 6, 5]:
                    emit_ctx(h, m_)

    nc.compile()   # bacc passes: split sync waits, move matmul waits to ldweights
    return nc


_nc_cache = {}


def _host_prep(x, attention_mask, Wq, bq, Wk, bk, Wv, bv, Wfc, bfc):
    """Host-side weight fusion (input-independent except x layout) ->
    (struct, nc, per-core in_maps, ybias)."""
    import ml_dtypes
    E4 = ml_dtypes.float8_e4m3

    x = np.asarray(x, np.float32)
    Wq64 = np.asarray(Wq, np.float64)
    Wk64 = np.asarray(Wk, np.float64)
    Wv64 = np.asarray(Wv, np.float64)
    Wfc64 = np.asarray(Wfc, np.float64)
    bq64 = np.asarray(bq, np.float64)
    bv64 = np.asarray(bv, np.float64)
    assert not bq64.any() and not np.asarray(bk, np.float64).any(), \
        "nonzero q/k bias not supported by this kernel variant"

    struct, ublocks = _mask_structure(attention_mask)
    key = struct[:3] + struct[4:]
    if key not in _nc_cache:
        _nc_cache[key] = _build(struct)
    nc = _nc_cache[key]

    scale = 1.0 / np.sqrt(np.float64(E))
    SH = 8.0      # w-hilo shift
    wa = np.empty((E, E * H), np.float64)
    wc = np.empty((E, E * H), np.float32)
    for g in range(H):
        gs = slice(g * E, (g + 1) * E)
        wa[:, gs] = scale * (Wk64[:, gs] @ Wq64[:, gs].T)
        wc[:, gs] = (Wv64[:, gs] @ Wfc64[gs, :]).astype(np.float32)
    wa8 = (wa * 256.0).astype(np.float32).astype(E4)
    # 3-term fp8 decomposition of w = x @ (64 C): see _build.emit_w
    chi8 = (wc * 64.0).astype(E4)
    chis8 = (wc * (64.0 / SH)).astype(E4)
    clos8 = ((wc * 64.0 - chi8.astype(np.float32)) * SH).astype(E4)
    ybias = (bv64 @ Wfc64 + np.asarray(bfc, np.float64)).astype(np.float32)

    # layout [identity, zero, mask blocks..., zero pad]: the mask matmul is a
    # DoubleRow pair (id, zero) x (block[uid], block[uid+1]) = 240*block[uid]
    em8 = np.concatenate(
        [240.0 * np.eye(P, dtype=np.float32)[None],
         np.zeros((1, P, P), np.float32),
         ublocks,
         np.zeros((1, P, P), np.float32)], axis=0).astype(E4)
    dcol = np.full((P, 2), 64.0, np.float32)   # psD rhs: w carries x64

    in_maps = []
    for c in range(NCORES):
        b, hg = divmod(c, 2)
        cs = slice(hg * EH, (hg + 1) * EH)
        xT = np.ascontiguousarray(x[b].T)
        xT8 = xT.astype(E4)
        in_maps.append({
            "xT8": xT8,
            "xhs8": (xT / SH).astype(E4),
            "xls8": ((xT - xT8.astype(np.float32)) * SH).astype(E4),
            "wa8": np.ascontiguousarray(wa8[:, cs]),
            "chi8": np.ascontiguousarray(chi8[:, cs]),
            "chis8": np.ascontiguousarray(chis8[:, cs]),
            "clos8": np.ascontiguousarray(clos8[:, cs]),
            "emask": em8,
            "dcol": dcol,
        })
    return key, nc, in_maps, ybias


def kernel(x, attention_mask, Wq, bq, Wk, bk, Wv, bv, Wfc, bfc, _trace=False):
    global LAST_RESULTS
    key, nc, in_maps, ybias = _host_prep(
        x, attention_mask, Wq, bq, Wk, bk, Wv, bv, Wfc, bfc)

    from concourse._compat import axon_active
    if axon_active() and not _trace:
        results = _run_pjrt_cached(key, nc, in_maps)
        LAST_RESULTS = None
    else:
        try:
            res = run_bass_kernel_spmd(nc, in_maps, core_ids=list(range(NCORES)),
                                       trace=_trace)
        except ModuleNotFoundError:
            # axon client without NTFF-profiling support: tracing disabled
            import os
            os.environ["BASS_NEVER_TRACE"] = "1"
            res = run_bass_kernel_spmd(nc, in_maps, core_ids=list(range(NCORES)),
                                       trace=False)
        LAST_RESULTS = res
        results = res.results
    out = np.empty((B, S, E), np.float32)
    for b in range(B):
        out[b] = results[2 * b]["y"] + results[2 * b + 1]["y"] + ybias
    return out


_jit_cache = {}


def _run_pjrt_cached(key, nc, in_maps):
    """bass2jax.run_bass_via_pjrt with the sharded jit cached per kernel
    structure, so repeated kernel() calls skip re-tracing (and with it the
    expensive NEFF recompile inside the neuronx_cc hook)."""
    import jax
    from jax.sharding import Mesh, PartitionSpec
    from jax.experimental.shard_map import shard_map
    from concourse import bass2jax
    import concourse.mybir as _mybir

    if key not in _jit_cache:
        bass2jax.install_neuronx_cc_hook()
        in_names, out_names, out_avals, zero_shapes = [], [], [], []
        for alloc in nc.m.functions[0].allocations:
            if not isinstance(alloc, _mybir.MemoryLocationSet):
                continue
            name = alloc.memorylocations[0].name
            if alloc.kind == "ExternalInput":
                if name != "partition_id":
                    in_names.append(name)
            elif alloc.kind == "ExternalOutput":
                shape = tuple(alloc.tensor_shape)
                dtype = _mybir.dt.np(alloc.dtype)
                out_names.append(name)
                out_avals.append(jax.core.ShapedArray(shape, dtype))
                zero_shapes.append((shape, dtype))
        n_params = len(in_names)
        n_outs = len(out_names)
        all_names = in_names + out_names + ["partition_id"]

        def _body(*args):
            operands = list(args)
            operands.append(bass2jax.partition_id_tensor())
            return tuple(bass2jax._bass_exec_p.bind(
                *operands,
                out_avals=tuple(out_avals),
                in_names=tuple(all_names),
                out_names=tuple(out_names),
                lowering_input_output_aliases=(),
                sim_require_finite=True,
                sim_require_nnan=True,
                nc=nc,
            ))

        devices = jax.devices()[:NCORES]
        mesh = Mesh(np.asarray(devices), ("core",))
        sharded = jax.jit(
            shard_map(_body, mesh=mesh,
                      in_specs=(PartitionSpec("core"),) * (n_params + n_outs),
                      out_specs=(PartitionSpec("core"),) * n_outs,
                      check_rep=False),
            donate_argnums=tuple(range(n_params, n_params + n_outs)),
            keep_unused=True,
        )
        _jit_cache[key] = (sharded, in_names, out_names, out_avals, zero_shapes)

    sharded, in_names, out_names, out_avals, zero_shapes = _jit_cache[key]
    concat_in = [
        np.concatenate([np.asarray(m[name]) for m in in_maps], axis=0)
        for name in in_names
    ]

    def _exec():
        concat_zeros = [np.zeros((NCORES * s[0], *s[1:]), d)
                        for s, d in zero_shapes]
        out_arrs = sharded(*concat_in, *concat_zeros)
        return [np.asarray(a) for a in out_arrs]

    try:
        out_arrs = _exec()
    except Exception:
        # transient device/transport flake: drop the failed call's effect
        # tokens (else jax's atexit block_until_ready re-raises even after a
        # successful retry) and retry once with fresh buffers
        try:
            from jax._src import dispatch as _jd
            _jd.runtime_tokens.clear()
        except Exception:
            pass
        out_arrs = _exec()
    return [
        {name: out_arrs[i].reshape(NCORES, *out_avals[i].shape)[c]
         for i, name in enumerate(out_names)}
        for c in range(NCORES)
    ]
